# revision 7
# baseline (speedup 1.0000x reference)
"""Entmax-1.5 over rows of a (2048, 32000) fp32 tensor on 8 Trainium2 NeuronCores.

Per row, with raw-units threshold c (y = relu((x - c)/2)^2, sum y = 1):
  1. SWDGE cast-DMA loads x as fp16 tiles (16000/8000/6000/2000 per 128-row
     block). Each tile folds independently (pairwise-halving strided max,
     groups of 16) to a G chunk; DVE max8 per subrange gives K=80 candidates.
  2. Warm Newton on candidates (fp32): free-window pre-iters on the first 48
     candidates, then 3 full iters on all 80 after the last tile lands.
  3. relu pass in place on DVE (tensor_scalar 4x fp16).
  4. f0 = sum (r/2)^2: block A entirely on ScalarE Square-accum (DVE is busy
     prepping block B); block B split: ScalarE tile-0 halves, DVE tiles 1-3
     via chained tensor_tensor_reduce (stt fallback via F0_STT=1).
  5. Newton on ScalarE: dc = max(0, (f0-1)*2*rs), nh = -dc/2.
  6. out pass: block A on ScalarE Square(0.5 r + nh) except tile-1 on DVE;
     block B split DVE (tiles 3,2,1 shift+self-mult in place) / ScalarE
     (tile-0 halves). DMA units to fp16 DRAM output as they become ready.

Host: shard rows 8 ways, gather, cast fp16 -> fp32.
"""

import os
import numpy as np

import concourse.bass as bass
import concourse.bacc as bacc
import concourse.mybir as mybir
from concourse.tile import TileContext
from concourse.bass_utils import run_bass_kernel_spmd

f32 = mybir.dt.float32
f16 = mybir.dt.float16
Alu = mybir.AluOpType
Act = mybir.ActivationFunctionType
AxX = mybir.AxisListType.X

ROWS_TOTAL = 2048
V = 32000
N_CORES = 8
ROWS_PER_CORE = ROWS_TOTAL // N_CORES  # 256
P = 128

TILES = [16000, 8000, 6000, 2000]     # per-block load tiles (sum 32000)
GW = [1000, 500, 375, 125]            # per-tile fold target widths (g16)
NRG = [4, 2, 3, 1]                    # max8 subranges per tile
CAND_OFF = [0, 32, 48, 72]            # candidate column offsets
K = 80
K_PRE = 48                            # candidates available after tiles 0-1
WARM_PRE = int(os.environ.get("WARM_PRE", "3"))
WARM_POST = int(os.environ.get("WARM_POST", "3"))
F0_STT = os.environ.get("F0_STT", "0") == "1"


class _Blk:
    pass


def build_kernel(nc: bass.Bass):
    x = nc.dram_tensor("x", [ROWS_PER_CORE, V], f32, kind="ExternalInput").ap()
    y = nc.dram_tensor("y", [ROWS_PER_CORE, V], f16, kind="ExternalOutput").ap()

    with TileContext(nc) as tc:
        with (
            tc.tile_pool(name="data", bufs=2) as dpool,
            tc.tile_pool(name="fold", bufs=1) as gpool,
            tc.tile_pool(name="ybuf", bufs=2) as ypool,
            tc.tile_pool(name="trash", bufs=1) as tpool,
            tc.tile_pool(name="small", bufs=2) as spool,
        ):
            def sm(tag, cols=1, dt=f32):
                return spool.tile([P, cols], dt, tag=tag, name=tag)

            z0 = spool.tile([P, 1], f32, tag="z0", name="z0", bufs=1)
            nc.vector.memset(z0, 0.0)
            zb = z0.to_broadcast([P, K])

            def new_block(b):
                s = _Blk()
                s.rows = slice(b * P, (b + 1) * P)
                s.xt = []
                return s

            def load(s, name):
                with nc.named_scope(f"load{name}"):
                    off = 0
                    for w in TILES:
                        xt = dpool.tile([P, w], f16, tag=f"xt{w}", name="xt")
                        s.xt.append(xt)
                        nc.gpsimd.dma_start(out=xt, in_=x[s.rows, off:off + w])
                        off += w

            def fold_tile(s, t, name):
                """Fold tile t by pairwise halving into G[:, :GW[t]], then
                max8 each subrange into VK."""
                with nc.named_scope(f"fold{name}{t}"):
                    G = s.G
                    w = TILES[t]
                    h = w // 2
                    nc.vector.tensor_tensor(out=G[:, 0:h], in0=s.xt[t][:, 0:h],
                                            in1=s.xt[t][:, h:w], op=Alu.max)
                    while h > GW[t]:
                        nh_ = h // 2
                        nc.vector.tensor_tensor(out=G[:, 0:nh_],
                                                in0=G[:, 0:nh_],
                                                in1=G[:, nh_:h], op=Alu.max)
                        h = nh_
                    W = GW[t] // NRG[t]
                    for i in range(NRG[t]):
                        o = CAND_OFF[t] + 8 * i
                        nc.vector.max(out=s.VK[:, o:o + 8],
                                      in_=G[:, W * i:W * (i + 1)])

            def warm_iters(s, width, iters, tag):
                VKf, rV, rV2 = s.VKf, s.rV, s.rV2
                S, Q, rs, u, C = s.S, s.Q, s.rs, s.u, s.C
                for _ in range(iters):
                    nc.vector.scalar_tensor_tensor(
                        out=rV[:, :width], in0=VKf[:, :width], scalar=C,
                        in1=zb[:, :width], op0=Alu.subtract, op1=Alu.max,
                        accum_out=S)
                    nc.vector.scalar_tensor_tensor(
                        out=rV2[:, :width], in0=rV[:, :width], scalar=1.0,
                        in1=rV[:, :width], op0=Alu.mult, op1=Alu.mult,
                        accum_out=Q)
                    nc.vector.reciprocal(rs, S)
                    nc.vector.scalar_tensor_tensor(
                        out=u, in0=Q, scalar=4.0, in1=rs,
                        op0=Alu.subtract, op1=Alu.mult)
                    nc.vector.scalar_tensor_tensor(
                        out=C, in0=u, scalar=0.5, in1=C,
                        op0=Alu.mult, op1=Alu.add)

            def warm_pre(s, name):
                with nc.named_scope(f"warmpre{name}"):
                    nc.vector.tensor_copy(s.VKf[:, :K_PRE], s.VK[:, :K_PRE])
                    vsum = sm("vsum")
                    nc.vector.tensor_reduce(out=vsum, in_=s.VKf[:, :K_PRE],
                                            axis=AxX, op=Alu.add)
                    nc.vector.tensor_scalar_mul(s.C, vsum, 1.0 / K_PRE)
                    warm_iters(s, K_PRE, WARM_PRE, name)

            def warm_post(s, name):
                with nc.named_scope(f"warm{name}"):
                    nc.vector.tensor_copy(s.VKf, s.VK)
                    warm_iters(s, K, WARM_POST, name)
                    nc.vector.tensor_scalar_mul(s.nrsig, s.rs, -1.0)

            def relu_unit(s, t, lo, w):
                sl = slice(lo, lo + w)
                nc.vector.tensor_scalar(
                    out=s.xt[t][:, sl], in0=s.xt[t][:, sl],
                    scalar1=s.C, scalar2=0.0,
                    op0=Alu.subtract, op1=Alu.max)

            def f0_sc_unit(s, t, lo, w, col):
                yb = ypool.tile([P, 8000], f16, tag="yb", name="yb")
                nc.scalar.activation(
                    out=yb[:, :w], in_=s.xt[t][:, lo:lo + w],
                    func=Act.Square, scale=0.5,
                    accum_out=s.f0c[:, col:col + 1])

            def f0_dve(s, name):
                """DVE square-accum on tiles 1-3, chained into s.fv."""
                with nc.named_scope(f"f0v{name}"):
                    if F0_STT:
                        for ui, t in enumerate((1, 2, 3)):
                            w = TILES[t]
                            tr = tpool.tile([P, 8000], f16, tag="tr",
                                            name="tr")
                            nc.vector.scalar_tensor_tensor(
                                out=tr[:, :w], in0=s.xt[t][:, :w], scalar=0.25,
                                in1=s.xt[t][:, :w], op0=Alu.mult, op1=Alu.mult,
                                accum_out=s.f0c[:, 2 + ui:3 + ui])
                        s.fv = None
                    else:
                        acc = 0.0
                        for ui, t in enumerate((1, 2, 3)):
                            w = TILES[t]
                            dummy = tpool.tile([P, 1], f32, tag=f"ttd{ui}",
                                               name="ttd")
                            fa = tpool.tile([P, 1], f32, tag=f"fv{ui}",
                                            name="fv")
                            nc.vector.tensor_tensor_reduce(
                                out=dummy.broadcast_to((P, w)),
                                in0=s.xt[t][:, :w], in1=s.xt[t][:, :w],
                                scale=0.25, scalar=acc,
                                op0=Alu.mult, op1=Alu.add,
                                accum_out=fa)
                            acc = fa
                        s.fv = acc

            def newton(s, name, ncols):
                with nc.named_scope(f"newt{name}"):
                    f0 = sm("f0")
                    if s.fv is not None:
                        nc.vector.tensor_copy(s.f0c[:, ncols:ncols + 1], s.fv)
                        ncols += 1
                    nc.vector.tensor_reduce(out=f0, in_=s.f0c[:, :ncols],
                                            axis=AxX, op=Alu.add)
                    dc0, dc, nh = sm("dc0"), sm("dc"), sm("nh")
                    nc.scalar.activation(out=dc0, in_=f0, func=Act.Identity,
                                         scale=s.rs, bias=s.nrsig)
                    nc.scalar.activation(out=dc, in_=dc0, func=Act.Relu,
                                         scale=2.0)
                    nc.scalar.activation(out=nh, in_=dc, func=Act.Identity,
                                         scale=-0.5)
                    s.dc, s.nh = dc, nh

            def out_sc_unit(s, t, lo, w):
                glo = sum(TILES[:t]) + lo
                yb = ypool.tile([P, 8000], f16, tag="yb", name="yb")
                nc.scalar.activation(out=yb[:, :w], in_=s.xt[t][:, lo:lo + w],
                                     func=Act.Square, scale=0.5, bias=s.nh)
                nc.sync.dma_start(out=y[s.rows, glo:glo + w], in_=yb[:, :w])

            def out_v_unit(s, t, lo, w):
                sl = slice(lo, lo + w)
                glo = sum(TILES[:t]) + lo
                nc.vector.tensor_scalar(
                    out=s.xt[t][:, sl], in0=s.xt[t][:, sl],
                    scalar1=s.dc, scalar2=0.5,
                    op0=Alu.subtract, op1=Alu.mult)
                nc.vector.tensor_tensor(
                    out=s.xt[t][:, sl], in0=s.xt[t][:, sl],
                    in1=s.xt[t][:, sl], op=Alu.mult)
                nc.sync.dma_start(out=y[s.rows, glo:glo + w],
                                  in_=s.xt[t][:, sl])

            def alloc_blk(s, name):
                s.G = gpool.tile([P, 8000], f16, tag="G", name="G")
                s.VK = spool.tile([P, K], f16, tag="VK", name="VK")
                s.VKf = sm("VKf", K)
                s.rV, s.rV2 = sm("rV", K), sm("rV2", K)
                s.S, s.Q, s.rs, s.u, s.C = (sm("S"), sm("Q"), sm("rs"),
                                            sm("u"), sm("C"))
                s.nrsig = sm("nrsig")
                s.f0c = sm("f0c", 6)
                s.fv = None

            A, B = new_block(0), new_block(1)
            load(A, "A")
            load(B, "B")

            # ---- block A threshold chain ----
            alloc_blk(A, "A")
            fold_tile(A, 0, "A")
            fold_tile(A, 1, "A")
            warm_pre(A, "A")
            fold_tile(A, 2, "A")
            fold_tile(A, 3, "A")
            warm_post(A, "A")
            with nc.named_scope("reluA"):
                for (t, lo, w) in [(0, 0, 8000), (0, 8000, 8000), (1, 0, 8000),
                                   (2, 0, 6000), (3, 0, 2000)]:
                    relu_unit(A, t, lo, w)
            # f0 A entirely on ScalarE (DVE preps block B meanwhile)
            with nc.named_scope("f0scA"):
                f0_sc_unit(A, 0, 0, 8000, 0)
                f0_sc_unit(A, 0, 8000, 8000, 1)
                f0_sc_unit(A, 1, 0, 8000, 2)
                f0_sc_unit(A, 2, 0, 6000, 3)
                f0_sc_unit(A, 3, 0, 2000, 4)
            newton(A, "A", 5)
            with nc.named_scope("outscA1"):
                out_sc_unit(A, 0, 0, 8000)       # u0
                out_sc_unit(A, 0, 8000, 8000)    # u1

            # ---- block B prep (DVE; interleaves with A work above) ----
            alloc_blk(B, "B")
            fold_tile(B, 0, "B")
            fold_tile(B, 1, "B")
            warm_pre(B, "B")
            fold_tile(B, 2, "B")
            fold_tile(B, 3, "B")
            warm_post(B, "B")
            with nc.named_scope("reluB"):
                for (t, lo, w) in [(0, 0, 8000), (0, 8000, 8000), (1, 0, 8000),
                                   (2, 0, 6000), (3, 0, 2000)]:
                    relu_unit(B, t, lo, w)
            # outA tile-1 on DVE (fits between reluB and f0B-dve)
            with nc.named_scope("outvA"):
                out_v_unit(A, 1, 0, 8000)
            # f0 B: ScalarE tile-0 halves, DVE tiles 1-3
            with nc.named_scope("f0scB"):
                f0_sc_unit(B, 0, 0, 8000, 0)
                f0_sc_unit(B, 0, 8000, 8000, 1)
            f0_dve(B, "B")
            # outA tiles 2,3 on ScalarE (after f0B scalar units)
            with nc.named_scope("outscA2"):
                out_sc_unit(A, 2, 0, 6000)
                out_sc_unit(A, 3, 0, 2000)
            newton(B, "B", 5 if F0_STT else 2)
            with nc.named_scope("outB"):
                out_v_unit(B, 3, 0, 2000)
                out_v_unit(B, 2, 0, 6000)
                out_sc_unit(B, 0, 0, 8000)
                out_v_unit(B, 1, 0, 8000)
                out_sc_unit(B, 0, 8000, 8000)
    return nc


_COMPILED = {}


def _get_nc():
    if "nc" not in _COMPILED:
        nc = bacc.Bacc("TRN2", target_bir_lowering=False, debug=False,
                       num_devices=N_CORES)
        build_kernel(nc)
        nc.compile()
        _COMPILED["nc"] = nc
    return _COMPILED["nc"]


def kernel(X: np.ndarray) -> np.ndarray:
    assert X.shape == (ROWS_TOTAL, V) and X.dtype == np.float32, (X.shape, X.dtype)
    nc = _get_nc()
    in_maps = [
        {"x": np.ascontiguousarray(X[i * ROWS_PER_CORE:(i + 1) * ROWS_PER_CORE])}
        for i in range(N_CORES)
    ]
    res = run_bass_kernel_spmd(nc, in_maps, core_ids=list(range(N_CORES)))
    return np.concatenate(
        [r["y"].astype(np.float32) for r in res.results], axis=0)
